# revision 1
# baseline (speedup 1.0000x reference)
"""Multi-head attention (dense transformer block) on 8 Trainium2 NeuronCores.

Sharding: one attention head per core (H=8 heads, 8 cores), both batch
elements on every core; QKV/O weights are sliced per head on the host and
each core computes its head's full attention plus its partial contribution
to the output projection. Host sums the 8 partial projections (the only
cross-core reduction; there is no device-to-device communication).

Per-core pipeline (all matmul inputs fp16, fp32 accumulation):
  q/k/vT[e,n] = W_head @ X_b^T                 (PE, contraction d=512)
  S^T[j,i]    = (K^T chunk).T-matmul(Q^T)      (PE, contraction hd=64)
  P^T         = exp(S^T/8 - SHIFT)             (ACT; SHIFT exact for softmax)
  Ou^T[d,i]   = V_aug.T @ P^T                  (PE; V_aug has a ones column
                                                so row 64 accumulates l=sum P)
  part[i,e]   = (Ou_aug chunk).T @ w_o_aug     (PE; w_o_aug row 64 = b_o so
                                                the bias enters as l*b_o)
  out         = part * (1/l)                   (DVE at evacuation)
"""
import numpy as np
from contextlib import ExitStack

import concourse.bass as bass
import concourse.tile as tile
from concourse import bacc, mybir
from concourse.bass_utils import run_bass_kernel_spmd
from concourse.masks import make_identity

dt = mybir.dt

H = 8
HD = 64
D = 512
B = 2
N = 4096
SCALE = 0.125
SHIFT = 2.0  # global logit shift; exact for softmax, keeps exp() in fp16 range

MM_DT = dt.float16
MM_NP = np.float16


def _build(repeat=1, ichunk=1024, ppool_bufs=34, spool_bufs=2, xt_bufs=12,
           exp_width=1024):
    # ichunk=1024 measured 552us/iter vs 634 at ichunk=512 (interleaved A/B,
    # same 6.603e-4 rel err): halves ACT exp-op count and i-chunk pass
    # overhead. PSUM: spool 2x[128,1024](4 banks) + opool 2 + projpool 2 = 8.
    NB = B * N
    njb = N // 128
    nic = N // ichunk
    mm_per_ic = ichunk // 512
    nexp = ichunk // exp_width

    nc = bacc.Bacc("TRN2", target_bir_lowering=False, debug=False, num_devices=8)
    xt = nc.dram_tensor("xt", [D, NB], MM_DT, kind="ExternalInput").ap()
    wqkvt = nc.dram_tensor("wqkvt", [D, 3 * HD], MM_DT, kind="ExternalInput").ap()
    woat = nc.dram_tensor("woat", [HD + 1, D], MM_DT, kind="ExternalInput").ap()
    part = nc.dram_tensor("part", [B, N, D], dt.float32, kind="ExternalOutput").ap()

    with tile.TileContext(nc) as tc:
        with ExitStack() as ctx:
            const_p = ctx.enter_context(tc.tile_pool(name="const", bufs=1))
            xt_p = ctx.enter_context(tc.tile_pool(name="xt", bufs=xt_bufs))
            qkv_p = ctx.enter_context(tc.tile_pool(name="qkv", bufs=1))
            vaug_p = ctx.enter_context(tc.tile_pool(name="vaug", bufs=1))
            pt_p = ctx.enter_context(tc.tile_pool(name="pt", bufs=ppool_bufs))
            out_p = ctx.enter_context(tc.tile_pool(name="outs", bufs=6))
            ot_p = ctx.enter_context(tc.tile_pool(name="ot", bufs=4))
            small_p = ctx.enter_context(tc.tile_pool(name="small", bufs=8))
            spool = ctx.enter_context(tc.tile_pool(name="spool", bufs=spool_bufs, space="PSUM"))
            opool = ctx.enter_context(tc.tile_pool(name="opool", bufs=2, space="PSUM"))
            projpool = ctx.enter_context(tc.tile_pool(name="projpool", bufs=2, space="PSUM"))

            ident = const_p.tile([128, 128], MM_DT, tag="ident")
            make_identity(nc, ident[:])
            shiftc = const_p.tile([128, 1], dt.float32, tag="shiftc")
            nc.vector.memset(shiftc[:], -SHIFT)
            wq = const_p.tile([128, 4, 3 * HD], MM_DT, tag="wq")
            for d in range(4):
                nc.sync.dma_start(wq[:, d, :], wqkvt[d * 128:(d + 1) * 128, :])
            woa = const_p.tile([HD + 1, D], MM_DT, tag="woa")
            nc.sync.dma_start(woa[:], woat[:])

            qT = qkv_p.tile([64, NB], MM_DT, tag="qT")
            kT = qkv_p.tile([64, NB], MM_DT, tag="kT")
            vT = qkv_p.tile([64, NB], MM_DT, tag="vT")
            vaug = [vaug_p.tile([128, njb * 65], MM_DT, tag=f"vaug{b}", name=f"vaug{b}")
                    for b in range(B)]

            def body(_=None):
                # ---- QKV projection ----
                for b in range(B):
                    for ch in range(N // 512):
                        c0 = b * N + ch * 512
                        xts = []
                        for d in range(4):
                            t = xt_p.tile([128, 512], MM_DT, tag="xt", name="xt_t")
                            nc.sync.dma_start(t[:], xt[d * 128:(d + 1) * 128, c0:c0 + 512])
                            xts.append(t)
                        ps_q = spool.tile([64, 512], dt.float32, tag="s", name="ps_q")
                        ps_k = spool.tile([64, 512], dt.float32, tag="s", name="ps_k")
                        ps_v = spool.tile([64, 512], dt.float32, tag="s", name="ps_v")
                        for d in range(4):
                            nc.tensor.matmul(ps_q[:], wq[:, d, 0:HD], xts[d][:],
                                             start=(d == 0), stop=(d == 3))
                        for d in range(4):
                            nc.tensor.matmul(ps_k[:], wq[:, d, HD:2 * HD], xts[d][:],
                                             start=(d == 0), stop=(d == 3))
                        for d in range(4):
                            nc.tensor.matmul(ps_v[:], wq[:, d, 2 * HD:3 * HD], xts[d][:],
                                             start=(d == 0), stop=(d == 3))
                        nc.vector.tensor_copy(qT[:, c0:c0 + 512], ps_q[:])
                        nc.vector.tensor_copy(kT[:, c0:c0 + 512], ps_k[:])
                        nc.vector.tensor_copy(vT[:, c0:c0 + 512], ps_v[:])

                # ---- V_aug (V transposed + ones column) ----
                for b in range(B):
                    nc.vector.memset(vaug[b][:], 1.0)
                    for jb in range(njb):
                        pt = opool.tile([128, 64], MM_DT, tag="o", name="pt_tr")
                        nc.tensor.transpose(
                            pt[:], vT[:, b * N + jb * 128: b * N + (jb + 1) * 128],
                            ident[0:64, 0:64])
                        nc.vector.tensor_copy(vaug[b][:, jb * 65: jb * 65 + 64], pt[:])

                # ---- attention + projection ----
                for b in range(B):
                    for ic in range(nic):
                        i0 = b * N + ic * ichunk
                        pts = []
                        for jb in range(njb):
                            ps_s = spool.tile([128, ichunk], dt.float32, tag="s", name="ps_s")
                            for m in range(mm_per_ic):
                                nc.tensor.matmul(
                                    ps_s[:, m * 512:(m + 1) * 512],
                                    kT[:, b * N + jb * 128: b * N + (jb + 1) * 128],
                                    qT[:, i0 + m * 512: i0 + (m + 1) * 512],
                                    start=True, stop=True)
                            ptile = pt_p.tile([128, ichunk], MM_DT, tag="pt", name="ptile")
                            for e in range(nexp):
                                nc.scalar.activation(
                                    ptile[:, e * exp_width:(e + 1) * exp_width],
                                    ps_s[:, e * exp_width:(e + 1) * exp_width],
                                    mybir.ActivationFunctionType.Exp,
                                    bias=shiftc[:, 0:1], scale=SCALE)
                            pts.append(ptile)
                        ps_o = [opool.tile([65, 512], dt.float32, tag="o", name="ps_o")
                                for _ in range(mm_per_ic)]
                        for jb in range(njb):
                            for m in range(mm_per_ic):
                                nc.tensor.matmul(
                                    ps_o[m][:],
                                    vaug[b][:, jb * 65:(jb + 1) * 65],
                                    pts[jb][:, m * 512:(m + 1) * 512],
                                    start=(jb == 0), stop=(jb == njb - 1))
                        for m in range(mm_per_ic):
                            ouT = ot_p.tile([65, 512], MM_DT, tag="ot", name="ouT")
                            nc.vector.tensor_copy(ouT[:], ps_o[m][:])
                            lrec = small_p.tile([65, 512], dt.float32, tag="lrec", name="lrec")
                            nc.vector.reciprocal(lrec[64:65, :], ps_o[m][64:65, :])
                            lrecT = small_p.tile([128, 4], dt.float32, tag="lrecT", name="lrecT")
                            for ib in range(4):
                                nc.sync.dma_start(
                                    lrecT[:, ib:ib + 1],
                                    lrec[64:65, ib * 128:(ib + 1) * 128])
                            for ib in range(4):
                                ps_p = projpool.tile([128, 512], dt.float32, tag="pj", name="ps_p")
                                nc.tensor.matmul(ps_p[:], ouT[:, ib * 128:(ib + 1) * 128],
                                                 woa[:], start=True, stop=True)
                                osb = out_p.tile([128, 512], dt.float32, tag="ou", name="osb")
                                nc.vector.tensor_scalar_mul(osb[:], ps_p[:], lrecT[:, ib:ib + 1])
                                row0 = ic * ichunk + m * 512 + ib * 128
                                nc.sync.dma_start(part[b, row0:row0 + 128, :], osb[:])

            if repeat == 1:
                body()
            else:
                with tc.For_i(0, repeat, 1) as _i:
                    body()

    nc.compile()
    return nc


def _make_in_maps(x, w_qkv, w_o, b_o):
    xt = np.ascontiguousarray(x.transpose(2, 1, 0).reshape(D, B * N)).astype(MM_NP)
    in_maps = []
    for c in range(8):
        wqs = w_qkv[c * HD:(c + 1) * HD]
        wks = w_qkv[D + c * HD:D + (c + 1) * HD]
        wvs = w_qkv[2 * D + c * HD:2 * D + (c + 1) * HD]
        wqkvt = np.ascontiguousarray(np.concatenate([wqs, wks, wvs], 0).T).astype(MM_NP)
        bo_row = b_o if c == 0 else np.zeros_like(b_o)
        woat = np.concatenate(
            [w_o[:, c * HD:(c + 1) * HD].T, bo_row[None, :]], 0).astype(MM_NP)
        in_maps.append({"xt": xt, "wqkvt": wqkvt, "woat": woat})
    return in_maps


_NC_CACHE = {}


def _get_nc(repeat=1, **kw):
    key = (repeat, tuple(sorted(kw.items())))
    if key not in _NC_CACHE:
        _NC_CACHE[key] = _build(repeat=repeat, **kw)
    return _NC_CACHE[key]


def kernel(x, w_qkv, w_o, b_o):
    x = np.asarray(x, np.float32)
    w_qkv = np.asarray(w_qkv, np.float32)
    w_o = np.asarray(w_o, np.float32)
    b_o = np.asarray(b_o, np.float32)
    assert x.shape == (N, B, D), x.shape
    nc = _get_nc()
    in_maps = _make_in_maps(x, w_qkv, w_o, b_o)
    res = run_bass_kernel_spmd(nc, in_maps, list(range(8)))
    acc = np.zeros((B, N, D), np.float64)
    for r in res.results:
        acc += r["part"]
    return acc.astype(np.float32)



# revision 6
# speedup vs baseline: 1.6122x; 1.6122x over previous
"""Multi-head attention (dense transformer block) on 8 Trainium2 NeuronCores.

Sharding: one attention head per core (H=8 heads, 8 cores), both batch
elements on every core. Each core computes q/k/v projections for its head,
the full NxN softmax attention, and returns the 65-row augmented attention
output (64 V-dims + softmax-denominator row) per 512-query chunk. The host
applies the small output projection w_o (fp32), divides by the denominator,
and sums the 8 per-head partials (the only cross-core reduction; no
device-to-device communication).

Per-core pipeline (matmul inputs fp16, fp32 PSUM accumulation):
  QKV    ps[qq|kv] = [Wq|Wq],[Wk|Wv] d-chunks @ X^T     (PE, K=128, M=128)
  S^T    pair p: j-block p at array rows 0:64 and j-block p+16 at rows
         64:128 run CONCURRENTLY via tile_position row packing (K=hd=64
         each, so both fit the 128-row PE array -> 2x S throughput)
  P      split across engines: ACT exp(0.125*S + BA) -> fp16, and DVE
         fast-exp2: int16(round(C1*S + C2)) bit-cast to fp16 (Schraudolph),
         one fused tensor_scalar op. C2 shifts the global exp scale so the
         int16 value is always positive; V is pre-scaled by 2^-G on the
         host so fp16 Ou can't overflow; the scale cancels exactly in the
         host-side division by the returned denominator row.
  Ou^T   [V^T | 2^-G ones] @ P^T accumulated over 32 j-blocks (M=65; row 64
         accumulates l = 2^-G * sum P)
  out    single [65,512] fp32 copy PSUM->SBUF, DMA to DRAM (2.1 MB/core)
"""
import numpy as np
from contextlib import ExitStack

import concourse.bass as bass
import concourse.tile as tile
from concourse import bacc, mybir
from concourse.bass_utils import run_bass_kernel_spmd
from concourse.masks import make_identity

dt = mybir.dt

H = 8
HD = 64
D = 512
B = 2
N = 4096
NB = B * N          # 8192
NCH = NB // 512     # 16 chunks of 512 queries
NPAIR = 16          # 32 j-blocks of 128 as 16 row-packed pairs

LOG2E = 1.4426950408889634
C1 = 0.125 * LOG2E * 1024.0
C2 = 14932.0
BA = (C2 - 15360.0) / 1024.0 * float(np.log(2.0))  # ACT-path logit offset
G = 4                                              # V pre-scale 2^-G

MM_DT = dt.float16
MM_NP = np.float16

# evenly spread DVE-exp pair assignments by count
DVE_SETS = {0: (), 4: (2, 6, 10, 14), 5: (1, 4, 7, 10, 13),
            6: (1, 4, 6, 9, 12, 14), 7: (1, 3, 5, 8, 10, 12, 14),
            8: (0, 2, 4, 6, 8, 10, 12, 14), 10: (0, 1, 3, 4, 6, 8, 9, 11, 12, 14)}


def _build(repeat=1, ndve=6, spair_bufs=3, pt_bufs=10, xt_bufs=8, ot_bufs=4):
    dve_set = set(DVE_SETS[ndve])
    nc = bacc.Bacc("TRN2", target_bir_lowering=False, debug=False, num_devices=8)
    xt = nc.dram_tensor("xt", [D, NB], MM_DT, kind="ExternalInput").ap()
    wa = nc.dram_tensor("wa", [D, 128], MM_DT, kind="ExternalInput").ap()
    wb = nc.dram_tensor("wb", [D, 256], MM_DT, kind="ExternalInput").ap()
    oub = nc.dram_tensor("oub", [NCH, HD + 1, 512], dt.float32,
                         kind="ExternalOutput").ap()

    with tile.TileContext(nc) as tc:
        with ExitStack() as ctx:
            const_p = ctx.enter_context(tc.tile_pool(name="const", bufs=1))
            persist = ctx.enter_context(tc.tile_pool(name="persist", bufs=1))
            xt_p = ctx.enter_context(tc.tile_pool(name="xtp", bufs=xt_bufs))
            pt_p = ctx.enter_context(tc.tile_pool(name="ptp", bufs=pt_bufs))
            ot_p = ctx.enter_context(tc.tile_pool(name="otp", bufs=ot_bufs))
            spool = ctx.enter_context(
                tc.tile_pool(name="spool", bufs=spair_bufs, space="PSUM"))
            opool = ctx.enter_context(
                tc.tile_pool(name="opool", bufs=2, space="PSUM"))

            ident = const_p.tile([128, 128], MM_DT, tag="ident")
            make_identity(nc, ident[:])
            actbias = const_p.tile([128, 1], dt.float32, tag="actbias")
            nc.vector.memset(actbias[:], BA)
            wq_s = const_p.tile([128, 4, 128], MM_DT, tag="wq")
            wb_s = const_p.tile([128, 4, 256], MM_DT, tag="wb")
            for d in range(4):
                nc.sync.dma_start(wq_s[:, d, :], wa[d * 128:(d + 1) * 128, :])
                nc.sync.dma_start(wb_s[:, d, :], wb[d * 128:(d + 1) * 128, :])

            # persistent activation layouts (see module docstring)
            qdup = persist.tile([128, NB], MM_DT, tag="qdup")
            kpack = persist.tile([128, NB // 2], MM_DT, tag="kpack")
            vtmp = persist.tile([128, NB // 2], MM_DT, tag="vtmp")
            vaug = [persist.tile([128, NPAIR * 130], MM_DT, tag=f"vaug{b}",
                                 name=f"vaug{b}") for b in range(B)]
            for b in range(B):
                nc.vector.memset(vaug[b][:], 2.0 ** (-G))

            def body(_=None):
                # ---- QKV projection ----
                for ci in range(NCH):
                    b, ic = ci // 8, ci % 8
                    half = ic // 4
                    col0 = b * 2048 + (ic % 4) * 512
                    xts = []
                    for d in range(4):
                        t = xt_p.tile([128, 512], MM_DT, tag="xt", name="xt_t")
                        nc.sync.dma_start(
                            t[:], xt[d * 128:(d + 1) * 128, ci * 512:(ci + 1) * 512])
                        xts.append(t)
                    ps = spool.tile([128, 1024], dt.float32, tag="spair", name="ps_qkv")
                    for d in range(4):
                        nc.tensor.matmul(ps[:, 0:512], wq_s[:, d, :], xts[d][:],
                                         start=(d == 0), stop=(d == 3))
                    for d in range(4):
                        nc.tensor.matmul(ps[:, 512:1024],
                                         wb_s[:, d, half * 128:half * 128 + 128],
                                         xts[d][:], start=(d == 0), stop=(d == 3))
                    nc.scalar.copy(qdup[:, ci * 512:(ci + 1) * 512], ps[:, 0:512])
                    if half == 0:   # psB = [k; v]
                        nc.vector.tensor_copy(kpack[0:64, col0:col0 + 512],
                                              ps[0:64, 512:1024])
                        nc.vector.tensor_copy(vtmp[64:128, col0:col0 + 512],
                                              ps[64:128, 512:1024])
                    else:           # psB = [v; k]
                        nc.vector.tensor_copy(vtmp[0:64, col0:col0 + 512],
                                              ps[0:64, 512:1024])
                        nc.vector.tensor_copy(kpack[64:128, col0:col0 + 512],
                                              ps[64:128, 512:1024])

                # ---- V transposes into vaug (pairs: j-block p+16 | p) ----
                for b in range(B):
                    for p in range(NPAIR):
                        ptr = opool.tile([128, 128], MM_DT, tag="o", name="ptr")
                        nc.tensor.transpose(
                            ptr[:], vtmp[:, b * 2048 + p * 128: b * 2048 + (p + 1) * 128],
                            ident[:])
                        src = ptr[:].rearrange("p (two f) -> p two f", two=2)
                        dst = vaug[b][:, p * 130:p * 130 + 130].rearrange(
                            "p (two f) -> p two f", two=2)[:, :, 0:64]
                        nc.vector.tensor_copy(dst, src)

                # ---- attention ----
                for ci in range(NCH):
                    b, ic = ci // 8, ci % 8
                    i0 = ci * 512
                    ps_o = opool.tile([65, 512], dt.float32, tag="o", name="ps_o")
                    for p in range(NPAIR):
                        sp = spool.tile([128, 1024], dt.float32, tag="spair", name="sp")
                        nc.tensor.matmul(
                            sp[:, 0:512],
                            kpack[0:64, b * 2048 + p * 128: b * 2048 + (p + 1) * 128],
                            qdup[0:64, i0:i0 + 512],
                            start=True, stop=True, tile_position=(0, 0))
                        nc.tensor.matmul(
                            sp[:, 512:1024],
                            kpack[64:128, b * 2048 + p * 128: b * 2048 + (p + 1) * 128],
                            qdup[64:128, i0:i0 + 512],
                            start=True, stop=True, tile_position=(64, 0))
                        pt = pt_p.tile([128, 1024], MM_DT, tag="pt", name="ptile")
                        if p in dve_set:
                            nc.vector.tensor_scalar(
                                pt[:].bitcast(dt.int16), sp[:], C1, C2,
                                mybir.AluOpType.mult, mybir.AluOpType.add)
                        else:
                            nc.scalar.activation(
                                pt[:], sp[:], mybir.ActivationFunctionType.Exp,
                                bias=actbias[:, 0:1], scale=0.125)
                        nc.tensor.matmul(ps_o[:],
                                         vaug[b][:, p * 130 + 65:p * 130 + 130],
                                         pt[:, 0:512],
                                         start=(p == 0), stop=False)
                        nc.tensor.matmul(ps_o[:],
                                         vaug[b][:, p * 130:p * 130 + 65],
                                         pt[:, 512:1024],
                                         start=False, stop=(p == NPAIR - 1))
                    out_t = ot_p.tile([HD + 1, 512], dt.float32, tag="ot", name="out_t")
                    nc.vector.tensor_copy(out_t[:], ps_o[:])
                    nc.sync.dma_start(oub[ci], out_t[:])

            if repeat == 1:
                body()
            else:
                with tc.For_i(0, repeat, 1) as _i:
                    body()

    nc.compile()
    return nc


def _make_in_maps(x, w_qkv, w_o, b_o):
    xtp = np.ascontiguousarray(
        x.transpose(2, 1, 0).reshape(D, NB)).astype(MM_NP)
    vs = np.float32(2.0 ** (-G))
    in_maps = []
    for c in range(H):
        wq = w_qkv[c * HD:(c + 1) * HD].T.astype(MM_NP)                    # [512, 64]
        wk = w_qkv[D + c * HD:D + (c + 1) * HD].T.astype(MM_NP)
        wv = (w_qkv[2 * D + c * HD:2 * D + (c + 1) * HD].T * vs).astype(MM_NP)
        wa = np.ascontiguousarray(np.concatenate([wq, wq], 1))             # [512, 128]
        wb = np.ascontiguousarray(np.concatenate([wk, wv, wv, wk], 1))     # [512, 256]
        in_maps.append({"xt": xtp, "wa": wa, "wb": wb})
    return in_maps


_NC_CACHE = {}


def _get_nc(repeat=1, **kw):
    key = (repeat, tuple(sorted(kw.items())))
    if key not in _NC_CACHE:
        _NC_CACHE[key] = _build(repeat=repeat, **kw)
    return _NC_CACHE[key]


def kernel(x, w_qkv, w_o, b_o):
    x = np.asarray(x, np.float32)
    w_qkv = np.asarray(w_qkv, np.float32)
    w_o = np.asarray(w_o, np.float32)
    b_o = np.asarray(b_o, np.float32)
    assert x.shape == (N, B, D), x.shape
    nc = _get_nc()
    in_maps = _make_in_maps(x, w_qkv, w_o, b_o)
    res = run_bass_kernel_spmd(nc, in_maps, list(range(8)))
    acc = np.zeros((B, N, D), np.float64)
    for c, r in enumerate(res.results):
        O = np.asarray(r["oub"], np.float32).reshape(B, 8, HD + 1, 512)
        woc = w_o[:, c * HD:(c + 1) * HD]                       # [512, 64]
        for b in range(B):
            ouT = O[b, :, 0:HD, :].transpose(0, 2, 1).reshape(N, HD)
            l = O[b, :, HD, :].reshape(N, 1)
            acc[b] += (ouT @ woc.T) / l
    return (acc + b_o[None, None, :]).astype(np.float32)


# revision 8
# speedup vs baseline: 1.6148x; 1.0016x over previous
"""Multi-head attention (dense transformer block) on 8 Trainium2 NeuronCores.

Sharding: one attention head per core (H=8 heads, 8 cores), both batch
elements on every core. Each core computes q/k/v projections for its head,
the full NxN softmax attention, and returns the 65-row augmented attention
output (64 V-dims + softmax-denominator row) per 512-query chunk. The host
applies the small output projection w_o (fp32), divides by the denominator,
and sums the 8 per-head partials (the only cross-core reduction; no
device-to-device communication).

Per-core pipeline (matmul inputs fp16, fp32 PSUM accumulation; all matmuls
at base partition 0 — tile_position row/col packing measured ~9x SLOWER on
HW, so it is deliberately not used):
  QKV    psA = [Wq|Wv] d-chunks @ X^T, psB = [Wk|Wv] @ X^T  (K=128, M=128)
         q -> qT[0:64], k -> kpack[0:64], v -> vtmp[64:128]
  V^T    transpose with half identity: out[128,64] = (vtmp rows 64:128)^T
  S^T    MM(K=64): kpack j-block stationary, qT chunk streaming; j-blocks
         2p,2p+1 fill one [128,1024] PSUM slab (2 banks)
  P      split across engines: ACT exp(0.125*S + BA) -> fp16, and DVE
         fast-exp2: int16(round(C1*S + C2)) bit-cast to fp16 (Schraudolph),
         one fused tensor_scalar op. C2 shifts the global exp scale so the
         int16 value is always positive; V is pre-scaled by 2^-G on the
         host so fp16 Ou can't overflow; the scale cancels exactly in the
         host-side division by the returned denominator row.
  Ou^T   [V^T | 2^-G ones] @ P^T accumulated over 32 j-blocks (M=65; row 64
         accumulates l = 2^-G * sum P)
  out    single [65,512] fp32 copy PSUM->SBUF, DMA to DRAM (2.1 MB/core)
"""
import numpy as np
from contextlib import ExitStack

import concourse.bass as bass
import concourse.tile as tile
from concourse import bacc, mybir
from concourse.bass_utils import run_bass_kernel_spmd
from concourse.masks import make_identity

dt = mybir.dt

H = 8
HD = 64
D = 512
B = 2
N = 4096
NB = B * N          # 8192
NCH = NB // 512     # 16 chunks of 512 queries
NPAIR = 16          # 32 j-blocks of 128, processed as 16 slab-pairs

LOG2E = 1.4426950408889634
C1 = 0.125 * LOG2E * 1024.0
C2 = 14932.0
BA = (C2 - 15360.0) / 1024.0 * float(np.log(2.0))  # ACT-path logit offset
G = 4                                              # V pre-scale 2^-G

MM_DT = dt.float16
MM_NP = np.float16

# evenly spread DVE-exp pair assignments by count
DVE_SETS = {0: (), 4: (2, 6, 10, 14), 5: (1, 4, 7, 10, 13),
            6: (1, 4, 6, 9, 12, 14), 7: (1, 3, 5, 8, 10, 12, 14),
            8: (0, 2, 4, 6, 8, 10, 12, 14), 10: (0, 1, 3, 4, 6, 8, 9, 11, 12, 14)}


def _build(repeat=1, ndve=6, spair_bufs=3, pt_bufs=10, xt_bufs=8, ot_bufs=4):
    dve_set = set(DVE_SETS[ndve])
    nc = bacc.Bacc("TRN2", target_bir_lowering=False, debug=False, num_devices=8)
    xt = nc.dram_tensor("xt", [D, NB], MM_DT, kind="ExternalInput").ap()
    wa = nc.dram_tensor("wa", [D, 128], MM_DT, kind="ExternalInput").ap()
    wb = nc.dram_tensor("wb", [D, 128], MM_DT, kind="ExternalInput").ap()
    oub = nc.dram_tensor("oub", [NCH, HD + 1, 512], dt.float32,
                         kind="ExternalOutput").ap()

    with tile.TileContext(nc) as tc:
        with ExitStack() as ctx:
            const_p = ctx.enter_context(tc.tile_pool(name="const", bufs=1))
            persist = ctx.enter_context(tc.tile_pool(name="persist", bufs=1))
            xt_p = ctx.enter_context(tc.tile_pool(name="xtp", bufs=xt_bufs))
            pt_p = ctx.enter_context(tc.tile_pool(name="ptp", bufs=pt_bufs))
            ot_p = ctx.enter_context(tc.tile_pool(name="otp", bufs=ot_bufs))
            spool = ctx.enter_context(
                tc.tile_pool(name="spool", bufs=spair_bufs, space="PSUM"))
            opool = ctx.enter_context(
                tc.tile_pool(name="opool", bufs=2, space="PSUM"))

            ident = const_p.tile([128, 128], MM_DT, tag="ident")
            make_identity(nc, ident[:])
            actbias = const_p.tile([128, 1], dt.float32, tag="actbias")
            nc.vector.memset(actbias[:], BA)
            wa_s = const_p.tile([128, 4, 128], MM_DT, tag="wa")
            wb_s = const_p.tile([128, 4, 128], MM_DT, tag="wb")
            for d in range(4):
                nc.sync.dma_start(wa_s[:, d, :], wa[d * 128:(d + 1) * 128, :])
                nc.sync.dma_start(wb_s[:, d, :], wb[d * 128:(d + 1) * 128, :])

            # persistent activation layouts (see module docstring)
            qT = persist.tile([64, NB], MM_DT, tag="qT")
            kpack = persist.tile([64, NB], MM_DT, tag="kpack")
            vtmp = persist.tile([128, NB], MM_DT, tag="vtmp")
            vaug = [persist.tile([128, 32 * 65], MM_DT, tag=f"vaug{b}",
                                 name=f"vaug{b}") for b in range(B)]
            for b in range(B):
                nc.vector.memset(vaug[b][:], 2.0 ** (-G))
            nc.vector.memset(vtmp[0:64, :], 0.0)

            def body(_=None):
                # ---- QKV projection ----
                for ci in range(NCH):
                    c0 = ci * 512
                    xts = []
                    for d in range(4):
                        t = xt_p.tile([128, 512], MM_DT, tag="xt", name="xt_t")
                        nc.sync.dma_start(
                            t[:], xt[d * 128:(d + 1) * 128, c0:c0 + 512])
                        xts.append(t)
                    ps = spool.tile([128, 1024], dt.float32, tag="spair", name="ps_qkv")
                    for d in range(4):
                        nc.tensor.matmul(ps[:, 0:512], wa_s[:, d, :], xts[d][:],
                                         start=(d == 0), stop=(d == 3))
                    for d in range(4):
                        nc.tensor.matmul(ps[:, 512:1024], wb_s[:, d, :], xts[d][:],
                                         start=(d == 0), stop=(d == 3))
                    nc.scalar.copy(qT[:, c0:c0 + 512], ps[0:64, 0:512])
                    nc.vector.tensor_copy(vtmp[64:128, c0:c0 + 512],
                                          ps[64:128, 0:512])
                    nc.vector.tensor_copy(kpack[:, c0:c0 + 512],
                                          ps[0:64, 512:1024])

                # ---- V transposes into vaug ----
                for b in range(B):
                    for jb in range(32):
                        w0 = b * N + jb * 128
                        ptr = opool.tile([128, 64], MM_DT, tag="o", name="ptr")
                        nc.tensor.transpose(ptr[:], vtmp[:, w0:w0 + 128],
                                            ident[:, 64:128])
                        nc.vector.tensor_copy(
                            vaug[b][:, jb * 65:jb * 65 + 64], ptr[:])

                # ---- attention ----
                for ci in range(NCH):
                    b = ci // 8
                    i0 = ci * 512
                    ps_o = opool.tile([65, 512], dt.float32, tag="o", name="ps_o")
                    for p in range(NPAIR):
                        j0 = b * N + 2 * p * 128
                        sp = spool.tile([128, 1024], dt.float32, tag="spair", name="sp")
                        nc.tensor.matmul(sp[:, 0:512], kpack[:, j0:j0 + 128],
                                         qT[:, i0:i0 + 512], start=True, stop=True)
                        nc.tensor.matmul(sp[:, 512:1024], kpack[:, j0 + 128:j0 + 256],
                                         qT[:, i0:i0 + 512], start=True, stop=True)
                        pt = pt_p.tile([128, 1024], MM_DT, tag="pt", name="ptile")
                        if p in dve_set:
                            nc.vector.tensor_scalar(
                                pt[:].bitcast(dt.int16), sp[:], C1, C2,
                                mybir.AluOpType.mult, mybir.AluOpType.add)
                        else:
                            nc.scalar.activation(
                                pt[:], sp[:], mybir.ActivationFunctionType.Exp,
                                bias=actbias[:, 0:1], scale=0.125)
                        jb = 2 * p
                        nc.tensor.matmul(ps_o[:],
                                         vaug[b][:, jb * 65:jb * 65 + 65],
                                         pt[:, 0:512],
                                         start=(p == 0), stop=False)
                        nc.tensor.matmul(ps_o[:],
                                         vaug[b][:, (jb + 1) * 65:(jb + 1) * 65 + 65],
                                         pt[:, 512:1024],
                                         start=False, stop=(p == NPAIR - 1))
                    out_t = ot_p.tile([HD + 1, 512], dt.float32, tag="ot", name="out_t")
                    nc.vector.tensor_copy(out_t[:], ps_o[:])
                    nc.sync.dma_start(oub[ci], out_t[:])

            if repeat == 1:
                body()
            else:
                with tc.For_i(0, repeat, 1) as _i:
                    body()

    nc.compile()
    return nc


def _make_in_maps(x, w_qkv, w_o, b_o):
    xtp = np.ascontiguousarray(
        x.transpose(2, 1, 0).reshape(D, NB)).astype(MM_NP)
    vs = np.float32(2.0 ** (-G))
    in_maps = []
    for c in range(H):
        wq = w_qkv[c * HD:(c + 1) * HD].T.astype(MM_NP)                    # [512, 64]
        wk = w_qkv[D + c * HD:D + (c + 1) * HD].T.astype(MM_NP)
        wv = (w_qkv[2 * D + c * HD:2 * D + (c + 1) * HD].T * vs).astype(MM_NP)
        wa = np.ascontiguousarray(np.concatenate([wq, wv], 1))             # [512, 128]
        wb = np.ascontiguousarray(np.concatenate([wk, wv], 1))             # [512, 128]
        in_maps.append({"xt": xtp, "wa": wa, "wb": wb})
    return in_maps


_NC_CACHE = {}


def _get_nc(repeat=1, **kw):
    key = (repeat, tuple(sorted(kw.items())))
    if key not in _NC_CACHE:
        _NC_CACHE[key] = _build(repeat=repeat, **kw)
    return _NC_CACHE[key]


def kernel(x, w_qkv, w_o, b_o):
    x = np.asarray(x, np.float32)
    w_qkv = np.asarray(w_qkv, np.float32)
    w_o = np.asarray(w_o, np.float32)
    b_o = np.asarray(b_o, np.float32)
    assert x.shape == (N, B, D), x.shape
    nc = _get_nc()
    in_maps = _make_in_maps(x, w_qkv, w_o, b_o)
    res = run_bass_kernel_spmd(nc, in_maps, list(range(8)))
    acc = np.zeros((B, N, D), np.float64)
    for c, r in enumerate(res.results):
        O = np.asarray(r["oub"], np.float32).reshape(B, 8, HD + 1, 512)
        woc = w_o[:, c * HD:(c + 1) * HD]                       # [512, 64]
        for b in range(B):
            ouT = O[b, :, 0:HD, :].transpose(0, 2, 1).reshape(N, HD)
            l = O[b, :, HD, :].reshape(N, 1)
            acc[b] += (ouT @ woc.T) / l
    return (acc + b_o[None, None, :]).astype(np.float32)
